# revision 1
# baseline (speedup 1.0000x reference)
"""Multi-head attention block (QKV proj + causal-multiplicative-mask softmax
attention + out proj + residual + LayerNorm) on 8 Trainium2 NeuronCores.

Sharding: tensor-parallel over heads. Each core computes 2 of the 16 heads
end-to-end (QKV projections for its 128 feature columns, full attention for
its heads over all batches, and its slice of the output projection), then a
per-batch ReduceScatter sums the partial projection outputs and hands each
core a contiguous row shard on which it applies residual + LayerNorm.

The multiplicative causal mask (zeros above the diagonal, NOT -inf) means
softmax weights for j > i are exp(0) = 1.  Those contributions are the
suffix-sums of V, which we fold into the attention matmuls analytically:
  - V is stored with a ones-column per head (vh65), so the unnormalized
    context matmul also accumulates the softmax denominator.
  - a strict-upper-triangular ones matmul adds the within-diagonal-block
    j > i contributions (both numerator and denominator count).
  - per-block column-sums of vh65 are combined with a static selection
    matrix to add the contributions of all fully-masked blocks above the
    diagonal block.
This halves the score/exp/AV work vs. computing the full S x S attention.
"""

import numpy as np
import ml_dtypes

import concourse.bacc as bacc
import concourse.bass as bass
import concourse.mybir as mybir
import concourse.tile as tile
from concourse.bass_utils import run_bass_kernel_spmd

BF16 = ml_dtypes.bfloat16
F32 = mybir.dt.float32
BF = mybir.dt.bfloat16

B, S, D = 4, 2048, 1024
H, HD = 16, 64
SCALE = float(HD) ** 0.5
LN_EPS = 1e-5

NCORES = 8
HPC = H // NCORES          # heads per core = 2
FPC = HPC * HD             # feature cols per core = 128
NB = S // 128              # 16 in-batch row blocks of 128
RPC = B * S // NCORES      # 1024 output rows per core
RPB = S // NCORES          # 256 output rows per core per batch

_CACHE = {}



def _ecopy(nc, eng, out, in_):
    if eng == "v":
        nc.vector.tensor_copy(out=out, in_=in_)
    else:
        nc.scalar.activation(out=out, in_=in_,
                             func=mybir.ActivationFunctionType.Copy)

def _build_nc():
    nc = bacc.Bacc("TRN2", target_bir_lowering=False, debug=False,
                   num_devices=NCORES)

    # ---- I/O ----
    xqt = nc.dram_tensor("xqt", [D, B * S], BF, kind="ExternalInput")
    xkt = nc.dram_tensor("xkt", [D, B * S], BF, kind="ExternalInput")
    xvt = nc.dram_tensor("xvt", [D, B * S], BF, kind="ExternalInput")
    wq = nc.dram_tensor("wq", [D, FPC], BF, kind="ExternalInput")
    wk = nc.dram_tensor("wk", [D, FPC], BF, kind="ExternalInput")
    wv = nc.dram_tensor("wv", [D, FPC], BF, kind="ExternalInput")
    wp = nc.dram_tensor("wp", [FPC, D], BF, kind="ExternalInput")
    bqs = nc.dram_tensor("bqs", [FPC, 1], F32, kind="ExternalInput")
    bks = nc.dram_tensor("bks", [FPC, 1], F32, kind="ExternalInput")
    bvs = nc.dram_tensor("bvs", [FPC, 1], F32, kind="ExternalInput")
    bp = nc.dram_tensor("bp", [1, D], F32, kind="ExternalInput")
    gam = nc.dram_tensor("gam", [1, D], F32, kind="ExternalInput")
    bet = nc.dram_tensor("bet", [1, D], F32, kind="ExternalInput")
    res = nc.dram_tensor("res", [RPC, D], F32, kind="ExternalInput")
    maskt = nc.dram_tensor("maskt", [128, 128], BF, kind="ExternalInput")
    triut = nc.dram_tensor("triut", [128, 128], BF, kind="ExternalInput")
    selb = nc.dram_tensor("selb", [NB, NB * 128], BF, kind="ExternalInput")
    ind16 = nc.dram_tensor("ind16", [128, NB * NB], BF, kind="ExternalInput")
    ident = nc.dram_tensor("ident", [128, 128], F32, kind="ExternalInput")
    y = nc.dram_tensor("y", [RPC, D], F32, kind="ExternalOutput")

    with tile.TileContext(nc) as tc:
        with tc.tile_pool(name="consts", bufs=1) as cpool, \
                tc.tile_pool(name="xbig", bufs=2) as xbig, \
                tc.tile_pool(name="kv", bufs=2) as kvp, \
                tc.tile_pool(name="exp", bufs=4) as epool, \
                tc.tile_pool(name="ctx", bufs=2) as ctxp, \
                tc.tile_pool(name="f32big", bufs=4) as fpool, \
                tc.tile_pool(name="small", bufs=3) as smallp, \
                tc.tile_pool(name="dram", bufs=2, space="DRAM") as dram, \
                tc.tile_pool(name="ps_scores", bufs=2, space="PSUM") as spool, \
                tc.tile_pool(name="ps_av", bufs=2, space="PSUM") as apool, \
                tc.tile_pool(name="ps_mm", bufs=2, space="PSUM") as ppool:
            # ---- constants ----
            def cload(src, shape, dtype, name):
                t = cpool.tile(shape, dtype, tag=name)
                nc.sync.dma_start(out=t[:], in_=src)
                return t

            wk_s = cload(wk.rearrange("(kk p) m -> p kk m", p=128),
                         [128, 8, FPC], BF, "wks")
            maskt_s = cload(maskt[:, :], [128, 128], BF, "maskt")
            triut_s = cload(triut[:, :], [128, 128], BF, "triut")
            selb_s = cload(selb[:, :], [NB, NB * 128], BF, "selb")
            ind16_s = cload(ind16[:, :], [128, NB * NB], BF, "ind16")
            ident_s = cload(ident[:, :], [128, 128], F32, "ident")
            bq_c = cload(bqs[:, :], [FPC, 1], F32, "bqc")
            bk_c = cload(bks[:, :], [FPC, 1], F32, "bkc")
            wq_s = cload(wq.rearrange("(kk p) m -> p kk m", p=128),
                         [128, 8, FPC], BF, "wqs")
            wv_s = cload(wv.rearrange("(kk p) m -> p kk m", p=128),
                         [128, 8, FPC], BF, "wvs")
            wp_s = cload(wp[:, :], [FPC, D], BF, "wps")
            bv_c = cload(bvs[:, :], [FPC, 1], F32, "bvc")
            bp_b = cload(bp[:, :].to_broadcast((128, D)), [128, D], F32, "bpb")
            gam_b = cload(gam[:, :].to_broadcast((128, D)), [128, D], F32, "gamb")
            bet_b = cload(bet[:, :].to_broadcast((128, D)), [128, D], F32, "betb")
            eps_c = cpool.tile([128, 1], F32, tag="eps")
            nc.vector.memset(eps_c[:], LN_EPS)

            rs_outs = [nc.dram_tensor(f"rsout{i}", [128, D], BF)
                       for i in range(6)]
            rs_q = [nc.dram_tensor(f"rsoutq{i}", [64, D], BF)
                    for i in range(4)]
            for b in range(B):
                bounce = dram.tile([S, D], BF, tag="bounce")
                bs = slice(b * S, (b + 1) * S)

                # ---- K projection -> khT_b [128 feat, 2048 rows] bf16 ----
                xk_h = []
                for hx in range(2):
                    xkh = xbig.tile([128, 8, S // 2], BF, tag="xk")
                    nc.sync.dma_start(
                        out=xkh[:],
                        in_=xkt[:, b * S + hx * 1024:b * S + (hx + 1) * 1024]
                        .rearrange("(kk p) n -> p kk n", p=128))
                    xk_h.append(xkh)
                khT_b = kvp.tile([FPC, S], BF, tag="khT")
                for n in range(4):
                    ps = ppool.tile([128, 512], F32, tag="mm")
                    for kk in range(8):
                        nc.tensor.matmul(
                            ps[:, :], wk_s[:, kk, :],
                            xk_h[n // 2][:, kk,
                                         (n % 2) * 512:(n % 2 + 1) * 512],
                            start=(kk == 0), stop=(kk == 7))
                    nc.vector.tensor_scalar(
                        out=khT_b[:, n * 512:(n + 1) * 512], in0=ps[:, :],
                        scalar1=bk_c[:, :], scalar2=None,
                        op0=mybir.AluOpType.add)

                # ---- V projection -> vhT (f32), then transpose per block ----
                xv_h = []
                for hx in range(2):
                    xvh = xbig.tile([128, 8, S // 2], BF, tag="xv")
                    nc.sync.dma_start(
                        out=xvh[:],
                        in_=xvt[:, b * S + hx * 1024:b * S + (hx + 1) * 1024]
                        .rearrange("(kk p) n -> p kk n", p=128))
                    xv_h.append(xvh)
                vhT_b = kvp.tile([FPC, S], F32, tag="vhT")
                for n in range(4):
                    ps = ppool.tile([128, 512], F32, tag="mm")
                    for kk in range(8):
                        nc.tensor.matmul(
                            ps[:, :], wv_s[:, kk, :],
                            xv_h[n // 2][:, kk,
                                         (n % 2) * 512:(n % 2 + 1) * 512],
                            start=(kk == 0), stop=(kk == 7))
                    nc.vector.tensor_scalar(
                        out=vhT_b[:, n * 512:(n + 1) * 512], in0=ps[:, :],
                        scalar1=bv_c[:, :], scalar2=None,
                        op0=mybir.AluOpType.add)
                vh65_b = kvp.tile([128, NB, 2 * 65], BF, tag="vh65")
                nc.vector.memset(
                    vh65_b[:].rearrange("p r (h c) -> p r h c", c=65)
                    [:, :, :, 64:65], 1.0)
                for rb in range(NB):
                    pst = ppool.tile([128, 512], F32, tag="mm")
                    nc.tensor.transpose(
                        pst[:, 0:FPC],
                        vhT_b[:, rb * 128:(rb + 1) * 128], ident_s[:])
                    _ecopy(nc, "v" if rb % 2 else "s",
                           vh65_b[:, rb, :]
                           .rearrange("p (h c) -> p h c", c=65)[:, :, 0:64],
                           pst[:, 0:FPC]
                           .rearrange("p (h c) -> p h c", c=64))

                # block column-sums of vh65 (for masked-region suffix sums)
                psc = ppool.tile([128, 512], F32, tag="mm")
                for rb in range(NB):
                    nc.tensor.matmul(
                        psc[0:NB, 0:130],
                        ind16_s[:, rb * NB:(rb + 1) * NB],
                        vh65_b[:, rb, :],
                        start=(rb == 0), stop=(rb == NB - 1))
                colsum_b = kvp.tile([NB, 2 * 65], BF, tag="colsum")
                nc.vector.tensor_copy(out=colsum_b[:], in_=psc[0:NB, 0:130])

                # ---- Q projection -> qhT_b [128 feat, 2048 rows] (x1/8) ----
                xq_h = []
                for hx in range(2):
                    xqh = xbig.tile([128, 8, S // 2], BF, tag="xq")
                    nc.sync.dma_start(
                        out=xqh[:],
                        in_=xqt[:, b * S + hx * 1024:b * S + (hx + 1) * 1024]
                        .rearrange("(kk p) n -> p kk n", p=128))
                    xq_h.append(xqh)
                qhT_b = kvp.tile([FPC, S], BF, tag="qhT")
                for n in range(4):
                    ps = ppool.tile([128, 512], F32, tag="mm")
                    for kk in range(8):
                        nc.tensor.matmul(
                            ps[:, :], wq_s[:, kk, :],
                            xq_h[n // 2][:, kk,
                                         (n % 2) * 512:(n % 2 + 1) * 512],
                            start=(kk == 0), stop=(kk == 7))
                    nc.vector.tensor_scalar(
                        out=qhT_b[:, n * 512:(n + 1) * 512], in0=ps[:, :],
                        scalar1=1.0 / SCALE, scalar2=bq_c[:, :],
                        op0=mybir.AluOpType.mult, op1=mybir.AluOpType.add)

                # ---- attention, four 128-row query blocks at a time ----
                for p in range(NB // 4):
                    qblk = [4 * p + t for t in range(4)]
                    ng = 4 * p + 4
                    ctxn = []
                    for t in range(4):
                        cx = ctxp.tile([128, FPC], F32, tag=f"cx{t}",
                                       name=f"cx{t}")
                        ctxn.append(cx)
                    for h01 in range(HPC):
                        hp = slice(h01 * 64, h01 * 64 + 64)
                        vs = slice(h01 * 65, h01 * 65 + 65)
                        # scoresT [kj, qi-quad 512], 2 key blocks per PSUM
                        # tile, exp'd to SBUF bf16
                        av = apool.tile([65, 512], F32, tag="av")
                        first_av = True
                        for ti in range((ng + 1) // 2):
                            gs = list(range(ti * 2, min(ng, ti * 2 + 2)))
                            w = len(gs) * 512
                            st = spool.tile([128, 1024], F32, tag="sc")
                            for g in gs:
                                col = (g - ti * 2) * 512
                                nc.tensor.matmul(
                                    st[:, col:col + 512],
                                    khT_b[hp, g * 128:(g + 1) * 128],
                                    qhT_b[hp, 4 * p * 128:4 * p * 128 + 512],
                                    start=True, stop=True)
                            et = epool.tile([128, 1024], BF, tag="et")
                            nc.scalar.activation(
                                out=et[:, 0:w], in_=st[:, 0:w],
                                func=mybir.ActivationFunctionType.Exp)
                            for g in gs:
                                col = (g - ti * 2) * 512
                                for qq in qblk:
                                    qc = col + (qq - 4 * p) * 128
                                    if qq == g:
                                        # diagonal block: keep kj <= qi and
                                        # set the j > i half to weight 1
                                        nc.vector.tensor_mul(
                                            out=et[:, qc:qc + 128],
                                            in0=et[:, qc:qc + 128],
                                            in1=maskt_s[:, :])
                                        nc.vector.tensor_add(
                                            out=et[:, qc:qc + 128],
                                            in0=et[:, qc:qc + 128],
                                            in1=triut_s[:, :])
                                    elif qq < g:
                                        # kj block fully above qi: weight 1
                                        nc.vector.memset(
                                            et[:, qc:qc + 128], 1.0)
                            for g in gs:
                                col = (g - ti * 2) * 512
                                nc.tensor.matmul(
                                    av[:, :], vh65_b[:, g, vs],
                                    et[:, col:col + 512],
                                    start=first_av, stop=False)
                                first_av = False
                        # fully-masked blocks above the quad: suffix sums
                        nc.tensor.matmul(
                            av[:, :], colsum_b[:, vs],
                            selb_s[:, 4 * p * 128:4 * p * 128 + 512],
                            start=False, stop=True)
                        # transpose av to [qi, 65] columns; per-partition
                        # reciprocal of the denominator, normalize
                        av_sb = smallp.tile([65, 512], F32, tag="avsb")
                        nc.vector.tensor_copy(out=av_sb[:], in_=av[:, :])
                        avT = ppool.tile([128, 512], F32, tag="mm")
                        for t in range(4):
                            nc.tensor.transpose(
                                avT[:, t * 65:(t + 1) * 65],
                                av_sb[:, t * 128:(t + 1) * 128],
                                ident_s[0:65, 0:65])
                        for t in range(4):
                            rcp = smallp.tile([128, 1], F32, tag="rcp")
                            nc.vector.reciprocal(
                                out=rcp[:],
                                in_=avT[:, t * 65 + 64:t * 65 + 65])
                            nc.vector.tensor_scalar(
                                out=ctxn[t][:, h01 * 64:(h01 + 1) * 64],
                                in0=avT[:, t * 65:t * 65 + 64],
                                scalar1=rcp[:, :], scalar2=None,
                                op0=mybir.AluOpType.mult)
                    # transpose ctx -> [feat, qi]; partial out-projection
                    for t in range(4):
                        ctp = ppool.tile([128, 512], F32, tag="mm")
                        nc.tensor.transpose(ctp[:, 0:FPC], ctxn[t][:],
                                            ident_s[:])
                        ctxT = ctxp.tile([FPC, 128], BF, tag="ctxT")
                        _ecopy(nc, "v" if t % 2 else "s", ctxT[:],
                               ctp[:, 0:FPC])
                        po = fpool.tile([128, D], BF, tag="po")
                        for n2 in range(2):
                            pp = ppool.tile([128, 512], F32, tag="mm")
                            nc.tensor.matmul(
                                pp[:, :], ctxT[:],
                                wp_s[:, n2 * 512:(n2 + 1) * 512],
                                start=True, stop=True)
                            _ecopy(nc, "v" if (t + n2) % 2 else "s",
                                   po[:, n2 * 512:(n2 + 1) * 512], pp[:, :])
                        qb = 4 * p + t
                        nc.sync.dma_start(
                            out=bounce[qb * 128:(qb + 1) * 128, :], in_=po[:])

                # ---- sum partials across cores; rank r gets its rows,
                # then residual + LayerNorm on this core's row shard.
                # The last batch reduces in quarters to shrink the tail. ----
                def rs_ln(in_rows, rso, out_row, pn):
                    nc.gpsimd.collective_compute(
                        "ReduceScatter", mybir.AluOpType.add,
                        replica_groups=[list(range(NCORES))],
                        ins=[bounce[in_rows[0]:in_rows[1], :].opt()],
                        outs=[rso[:].opt()])
                    ldb = fpool.tile([128, D], BF, tag="po", name="ldb")
                    nc.sync.dma_start(out=ldb[0:pn, :], in_=rso[:, :])
                    ld = fpool.tile([128, D], F32, tag="f4k", name="ld")
                    nc.vector.tensor_add(out=ld[0:pn, :], in0=ldb[0:pn, :],
                                         in1=bp_b[0:pn, :])
                    rs_t = fpool.tile([128, D], F32, tag="f4k", name="rs_t")
                    nc.sync.dma_start(
                        out=rs_t[0:pn, :],
                        in_=res[out_row:out_row + pn, :])
                    nc.vector.tensor_add(out=ld[0:pn, :], in0=ld[0:pn, :],
                                         in1=rs_t[0:pn, :])
                    stats = smallp.tile([128, 2, 6], F32, tag="stats",
                                        name="stats")
                    for c2 in range(2):
                        nc.vector.bn_stats(
                            out=stats[0:pn, c2, :],
                            in_=ld[0:pn, c2 * 512:(c2 + 1) * 512])
                    mv = smallp.tile([128, 2], F32, tag="mv", name="mv")
                    nc.vector.bn_aggr(out=mv[0:pn], in_=stats[0:pn])
                    sd = smallp.tile([128, 1], F32, tag="sd", name="sd")
                    nc.scalar.activation(
                        out=sd[0:pn], in_=mv[0:pn, 1:2],
                        func=mybir.ActivationFunctionType.Sqrt,
                        bias=eps_c[0:pn, :])
                    rstd = smallp.tile([128, 1], F32, tag="rstd",
                                       name="rstd")
                    nc.vector.reciprocal(out=rstd[0:pn], in_=sd[0:pn])
                    yt = fpool.tile([128, D], F32, tag="f4k", name="yt")
                    nc.vector.tensor_scalar(
                        out=yt[0:pn, :], in0=ld[0:pn, :],
                        scalar1=mv[0:pn, 0:1],
                        scalar2=rstd[0:pn, :], op0=mybir.AluOpType.subtract,
                        op1=mybir.AluOpType.mult)
                    nc.vector.tensor_mul(out=yt[0:pn, :], in0=yt[0:pn, :],
                                         in1=gam_b[0:pn, :])
                    nc.vector.tensor_add(out=yt[0:pn, :], in0=yt[0:pn, :],
                                         in1=bet_b[0:pn, :])
                    nc.sync.dma_start(
                        out=y[out_row:out_row + pn, :], in_=yt[0:pn, :])

                if b < B - 1:
                    for hf in range(2):
                        i8 = 2 * b + hf
                        rs_ln((hf * 1024, (hf + 1) * 1024), rs_outs[i8],
                              i8 * 128, 128)
                else:
                    for qt in range(4):
                        rs_ln((qt * 512, (qt + 1) * 512), rs_q[qt],
                              6 * 128 + qt * 64, 64)

    nc.compile()
    return nc


def _host_inputs(q, k, v, Wq, bq, Wk, bk, Wv, bv, Wp, bp, gamma, beta):
    """Build the 8 per-core input maps from the full-size inputs."""
    qf = np.asarray(q, np.float32).reshape(B * S, D)
    kf = np.asarray(k, np.float32).reshape(B * S, D)
    vf = np.asarray(v, np.float32).reshape(B * S, D)
    xqt = np.ascontiguousarray(qf.T).astype(BF16)
    xkt = np.ascontiguousarray(kf.T).astype(BF16)
    xvt = np.ascontiguousarray(vf.T).astype(BF16)

    Wq = np.asarray(Wq, np.float32)
    Wk = np.asarray(Wk, np.float32)
    Wv = np.asarray(Wv, np.float32)
    Wp = np.asarray(Wp, np.float32)
    bq = np.asarray(bq, np.float32)
    bk = np.asarray(bk, np.float32)
    bv = np.asarray(bv, np.float32)
    bp = np.asarray(bp, np.float32)
    gamma = np.asarray(gamma, np.float32)
    beta = np.asarray(beta, np.float32)

    ii, jj = np.meshgrid(np.arange(128), np.arange(128), indexing="ij")
    maskt = (ii <= jj).astype(BF16)          # [kj, qi]: keep j <= i
    triut = (ii > jj).astype(BF16)           # [kj, qi]: strict upper ones
    selb = np.zeros((NB, NB * 128), BF16)
    for p in range(NB // 4):
        selb[4 * p + 4:, p * 512:(p + 1) * 512] = 1
    ind16 = np.zeros((128, NB * NB), BF16)
    for rb in range(NB):
        ind16[:, rb * NB + rb] = 1
    ident = np.eye(128, dtype=np.float32)

    in_maps = []
    for r in range(NCORES):
        cs = slice(r * FPC, (r + 1) * FPC)
        rows = np.concatenate(
            [np.arange(b * S + hf * 1024 + r * 128,
                       b * S + hf * 1024 + (r + 1) * 128)
             for b in range(B - 1) for hf in range(2)] +
            [np.arange((B - 1) * S + qt * 512 + r * 64,
                       (B - 1) * S + qt * 512 + (r + 1) * 64)
             for qt in range(4)])
        in_maps.append({
            "xqt": xqt, "xkt": xkt, "xvt": xvt,
            "wq": Wq[:, cs].astype(BF16),
            "wk": Wk[:, cs].astype(BF16),
            "wv": Wv[:, cs].astype(BF16),
            "wp": np.ascontiguousarray(Wp[cs, :]).astype(BF16),
            "bqs": (bq[cs] / SCALE).reshape(FPC, 1).astype(np.float32),
            "bks": bk[cs].reshape(FPC, 1).astype(np.float32),
            "bvs": bv[cs].reshape(FPC, 1).astype(np.float32),
            "bp": bp.reshape(1, D),
            "gam": gamma.reshape(1, D),
            "bet": beta.reshape(1, D),
            "res": np.ascontiguousarray(qf[rows]),
            "maskt": maskt, "triut": triut, "selb": selb,
            "ind16": ind16, "ident": ident,
        })
    return in_maps


def _assemble(results):
    out = np.empty((B * S, D), np.float32)
    for r in range(NCORES):
        yr = results[r]["y"]
        for b in range(B - 1):
            for hf in range(2):
                g0 = b * S + hf * 1024 + r * 128
                l0 = (2 * b + hf) * 128
                out[g0:g0 + 128] = yr[l0:l0 + 128]
        for qt in range(4):
            g0 = (B - 1) * S + qt * 512 + r * 64
            l0 = 6 * 128 + qt * 64
            out[g0:g0 + 64] = yr[l0:l0 + 64]
    return out.reshape(B, S, D)


def kernel(**inputs) -> np.ndarray:
    if "nc" not in _CACHE:
        _CACHE["nc"] = _build_nc()
    nc = _CACHE["nc"]
    in_maps = _host_inputs(**inputs)
    res = run_bass_kernel_spmd(nc, in_maps, core_ids=list(range(NCORES)))
    return _assemble(res.results)


def kernel_profiled(**inputs):
    """Like kernel(), but captures an NTFF profile. Returns (out, result)."""
    if "nc" not in _CACHE:
        _CACHE["nc"] = _build_nc()
    nc = _CACHE["nc"]
    in_maps = _host_inputs(**inputs)
    res = run_bass_kernel_spmd(nc, in_maps, core_ids=list(range(NCORES)),
                               trace=True)
    return _assemble(res.results), res


if __name__ == "__main__":
    rng = np.random.default_rng(0)
    std = 1.0 / np.sqrt(D)
    inp = {
        "q": rng.standard_normal((B, S, D), np.float32),
        "k": rng.standard_normal((B, S, D), np.float32),
        "v": rng.standard_normal((B, S, D), np.float32),
        "Wq": rng.standard_normal((D, D), np.float32) * std,
        "bq": np.zeros(D, np.float32),
        "Wk": rng.standard_normal((D, D), np.float32) * std,
        "bk": np.zeros(D, np.float32),
        "Wv": rng.standard_normal((D, D), np.float32) * std,
        "bv": np.zeros(D, np.float32),
        "Wp": rng.standard_normal((D, D), np.float32) * std,
        "bp": np.zeros(D, np.float32),
        "gamma": np.ones(D, np.float32),
        "beta": np.zeros(D, np.float32),
    }
    out = kernel(**inp)
    print("kernel output:", out.shape, out.dtype)

